# revision 1
# baseline (speedup 1.0000x reference)
"""Trainium2 kernel for nn_BernoulliIndependentGenerator.

Strategy (data-parallel over batch, per sharding hint):
  - Host: embedding gather (index manipulation only).
  - Device (8 NeuronCores, 2 samples/core): the FLOP-heavy input
    projections xp = emb @ [w_ih_f | w_ih_b].T as tiled fp32 matmuls.
  - Host: sequential BiLSTM scan (1024 steps), gate scores, per-row
    top-k -> binary mask. Backward direction handled by shifting each
    sample's valid prefix to the end of the buffer so an unmasked
    reverse scan reproduces packed-sequence semantics.
"""

import numpy as np

B, S, E, H, V = 16, 1024, 256, 256, 50257
FOUR_H = 4 * H          # 1024
N_CORES = 8
BPC = B // N_CORES      # samples per core = 2
TOK = BPC * S           # tokens per core = 2048
BUDGET = 10


def _build_nc():
    import concourse.bass as bass
    import concourse.mybir as mybir
    from concourse.tile import TileContext

    nc = bass.Bass("TRN2")
    # packed input: [128, 8192] = [embT_k0 | embT_k1 | w_k0 | w_k1] blocks of 2048 cols
    inp = nc.dram_tensor("inp", [128, 4 * 2048], mybir.dt.float32, kind="ExternalInput")
    out = nc.dram_tensor("out", [TOK, 2 * FOUR_H], mybir.dt.float32, kind="ExternalOutput")

    KT = E // 128          # 2 k-tiles
    MT = TOK // 128        # 16 token tiles
    NT = (2 * FOUR_H) // 512  # 4 n-tiles of 512

    with TileContext(nc) as tc:
        with (
            tc.tile_pool(name="const", bufs=1) as cpool,
            tc.tile_pool(name="psum", bufs=8, space="PSUM") as ppool,
        ):
            big = cpool.tile([128, 4 * 2048], mybir.dt.float32, tag="inp")
            nc.gpsimd.dma_start(big[:], inp[:, :])
            st_all = cpool.tile([128, MT * 2 * FOUR_H], mybir.dt.float32, tag="st")

            for m in range(MT):
                for n in range(NT):
                    ps = ppool.tile([128, 512], mybir.dt.float32)
                    for k in range(KT):
                        nc.tensor.matmul(
                            ps[:],
                            big[:, k * 2048 + m * 128:k * 2048 + (m + 1) * 128],
                            big[:, 4096 + k * 2048 + n * 512:4096 + k * 2048 + (n + 1) * 512],
                            start=(k == 0),
                            stop=(k == KT - 1),
                        )
                    nc.vector.tensor_copy(
                        st_all[:, m * 2048 + n * 512:m * 2048 + (n + 1) * 512], ps[:]
                    )
            out_v = out.rearrange("(m p) c -> p m c", p=128)      # [128, 16, 2048]
            st_v = st_all[:].rearrange("p (m c) -> p m c", c=2048)
            nc.sync.dma_start(out_v, st_v)
    return nc


_NC_CACHE = None


def _device_projections(emb):
    """emb: [B, S, E] f32 -> xp [B, S, 2*4H] f32 (fwd cols 0:1024, bwd 1024:2048).
    Falls back to numpy matmul if the device path is unavailable."""
    global _NC_CACHE
    w_cat = _device_projections._w_cat  # [E, 2*4H] f32
    import os
    import signal

    if os.environ.get("KERNEL_NO_DEVICE"):
        return (emb.reshape(B * S, E) @ w_cat).reshape(B, S, 2 * FOUR_H)

    def _alarm(signum, frame):
        raise TimeoutError("device path timed out")

    old = None
    try:
        old = signal.signal(signal.SIGALRM, _alarm)
        signal.alarm(240)
    except Exception:
        old = None
    try:
        from concourse.bass_utils import run_bass_kernel_spmd

        if _NC_CACHE is None:
            _NC_CACHE = _build_nc()
        nc = _NC_CACHE
        in_maps = []
        w_pack = np.concatenate([w_cat[0:128, :], w_cat[128:256, :]], axis=1)
        for i in range(N_CORES):
            embT_i = emb[i * BPC:(i + 1) * BPC].reshape(TOK, E).T.astype(np.float32)
            packed = np.ascontiguousarray(
                np.concatenate(
                    [embT_i[0:128, :], embT_i[128:256, :], w_pack], axis=1
                )
            )
            in_maps.append({"inp": packed})
        res = run_bass_kernel_spmd(nc, in_maps, core_ids=list(range(N_CORES)))
        xp = np.empty((B, S, 2 * FOUR_H), np.float32)
        for i in range(N_CORES):
            xp[i * BPC:(i + 1) * BPC] = res.results[i]["out"].reshape(
                BPC, S, 2 * FOUR_H
            )
        return xp
    except Exception:
        # device path unavailable: equivalent host computation
        return (emb.reshape(B * S, E) @ w_cat).reshape(B, S, 2 * FOUR_H)
    finally:
        try:
            signal.alarm(0)
            if old is not None:
                signal.signal(signal.SIGALRM, old)
        except Exception:
            pass


def _sigmoid(x):
    return 1.0 / (1.0 + np.exp(-x))


def _scan(xp, w_hh_T, reverse):
    """Unmasked LSTM scan. xp: [B, S, 4H] f32, w_hh_T: [H, 4H]. Returns h: [B, S, H]."""
    Bn, Sn, _ = xp.shape
    h = np.zeros((Bn, H), np.float32)
    c = np.zeros((Bn, H), np.float32)
    hs = np.empty((Bn, Sn, H), np.float32)
    order = range(Sn - 1, -1, -1) if reverse else range(Sn)
    for t in order:
        gates = xp[:, t, :] + h @ w_hh_T
        i = _sigmoid(gates[:, 0:H])
        f = _sigmoid(gates[:, H:2 * H])
        g = np.tanh(gates[:, 2 * H:3 * H])
        o = _sigmoid(gates[:, 3 * H:4 * H])
        c = f * c + i * g
        h = o * np.tanh(c)
        hs[:, t, :] = h
    return hs


def kernel(**inputs):
    x = np.asarray(inputs["x"]).astype(np.int64)
    mask = np.asarray(inputs["mask"]).astype(bool)
    embed_table = np.asarray(inputs["embed_table"], dtype=np.float32)
    w_ih_f = np.asarray(inputs["w_ih_f"], dtype=np.float32)
    w_hh_f = np.asarray(inputs["w_hh_f"], dtype=np.float32)
    b_f = np.asarray(inputs["b_f"], dtype=np.float32)
    w_ih_b = np.asarray(inputs["w_ih_b"], dtype=np.float32)
    w_hh_b = np.asarray(inputs["w_hh_b"], dtype=np.float32)
    b_b = np.asarray(inputs["b_b"], dtype=np.float32)
    z_w = np.asarray(inputs["z_w"], dtype=np.float32)
    z_b = np.float32(np.asarray(inputs["z_b"]))

    lengths = mask.sum(1).astype(np.int64)            # [B]

    # ---- device: input projections for both directions ----
    _device_projections._w_cat = np.ascontiguousarray(
        np.concatenate([w_ih_f.T, w_ih_b.T], axis=1)
    ).astype(np.float32)                               # [E, 2048]
    emb = embed_table[x]                               # [B, S, E]
    xp = _device_projections(emb)
    xp_f = xp[:, :, :FOUR_H] + b_f                     # [B, S, 4H]
    xp_b = xp[:, :, FOUR_H:] + b_b

    # ---- host: BiLSTM scan (packed-sequence semantics via prefix shift) ----
    h_f = _scan(xp_f, np.ascontiguousarray(w_hh_f.T), reverse=False)

    # shift each sample's valid prefix to the END, reverse-scan unmasked,
    # then shift back: h_b[b, t] = h_b_shifted[b, t + S - L_b]
    shift = (S - lengths)                              # [B]
    rows = np.arange(S)[None, :]                       # [1, S]
    src = rows - shift[:, None]                        # shifted[t] = orig[src]
    src_c = np.clip(src, 0, S - 1)
    gather_idx = src_c[:, :, None]
    xp_b_shifted = np.take_along_axis(xp_b, np.broadcast_to(gather_idx, xp_b.shape), axis=1)
    xp_b_shifted = np.where((src >= 0)[:, :, None], xp_b_shifted, 0.0).astype(np.float32)
    h_b_shifted = _scan(xp_b_shifted, np.ascontiguousarray(w_hh_b.T), reverse=True)
    dst = rows + shift[:, None]                        # h_b[t] = shifted[dst]
    dst_c = np.clip(dst, 0, S - 1)
    h_b = np.take_along_axis(
        h_b_shifted, np.broadcast_to(dst_c[:, :, None], h_b_shifted.shape), axis=1
    )
    h_b = np.where((dst < S)[:, :, None], h_b, 0.0).astype(np.float32)

    # ---- gate scores + per-row top-k ----
    scores = h_f @ z_w[:H] + h_b @ z_w[H:] + z_b       # [B, S]
    probs = _sigmoid(scores.astype(np.float32))
    probs = np.where(mask, probs, 0.0).astype(np.float32)
    k = np.round(BUDGET / 100.0 * lengths.astype(np.float32)).astype(np.int64)
    ranks = np.argsort(np.argsort(-probs, axis=1, kind="stable"), axis=1, kind="stable")
    z = ((ranks < k[:, None]) & (probs > 0)).astype(np.float32)
    z = np.where(mask, z, 0.0).astype(np.float32)
    return z



# revision 3
# speedup vs baseline: 5.0376x; 5.0376x over previous
"""Trainium2 kernel for nn_BernoulliIndependentGenerator.

Full-device pipeline: per-core Bass program computes input projections,
the BiLSTM recurrence (fwd+bwd in one 1024-step hardware loop), and the
gate-score dot products for 2 samples; 8 cores cover B=16 (data
parallel over batch, per the sharding hint). Host does the embedding
gather, input packing, and the final sigmoid + per-row top-k.

Heavy one-time setup (imports, Bass trace, NEFF compile, executable
load, warm-up dispatch) happens at module import; kernel() itself only
packs inputs, runs one dispatch, and post-processes.
"""

import os
import numpy as np

B, S, E, H, V = 16, 1024, 256, 256, 50257
BUDGET = 10
N_CORES = 8
FOUR_H = 4 * H

# ---------------------------------------------------------------------------
# device setup (import time)
# ---------------------------------------------------------------------------

_DEV = None


def _init_device():
    import jax
    from jax.sharding import Mesh, PartitionSpec
    from jax.experimental.shard_map import shard_map
    import concourse.mybir as mybir
    from concourse.bass2jax import (
        install_neuronx_cc_hook, _bass_exec_p, partition_id_tensor,
    )
    from lstm_bass import build_core

    install_neuronx_cc_hook()
    nc = build_core(S)

    pname = nc.partition_id_tensor.name if nc.partition_id_tensor else None
    in_names, out_names, out_avals, zero_outs = [], [], [], []
    for alloc in nc.m.functions[0].allocations:
        if not isinstance(alloc, mybir.MemoryLocationSet):
            continue
        name = alloc.memorylocations[0].name
        if alloc.kind == "ExternalInput":
            if name != pname:
                in_names.append(name)
        elif alloc.kind == "ExternalOutput":
            out_names.append(name)
            shape = tuple(alloc.tensor_shape)
            dtype = mybir.dt.np(alloc.dtype)
            out_avals.append(jax.core.ShapedArray(shape, dtype))
            zero_outs.append((shape, dtype))
    n_params = len(in_names)
    n_outs = len(out_avals)
    in_names_all = list(in_names) + out_names + ([pname] if pname else [])
    donate = tuple(range(n_params, n_params + n_outs))

    def _body(*args):
        operands = list(args)
        if pname is not None:
            operands.append(partition_id_tensor())
        outs = _bass_exec_p.bind(
            *operands, out_avals=tuple(out_avals), in_names=tuple(in_names_all),
            out_names=tuple(out_names), lowering_input_output_aliases=(),
            sim_require_finite=False, sim_require_nnan=False, nc=nc)
        return tuple(outs)

    devices = jax.devices()[:N_CORES]
    mesh = Mesh(np.asarray(devices), ("core",))
    sharded = jax.jit(
        shard_map(_body, mesh=mesh,
                  in_specs=(PartitionSpec("core"),) * (n_params + n_outs),
                  out_specs=(PartitionSpec("core"),) * n_outs, check_rep=False),
        donate_argnums=donate, keep_unused=True)

    shapes = {}
    for alloc in nc.m.functions[0].allocations:
        if not isinstance(alloc, mybir.MemoryLocationSet):
            continue
        name = alloc.memorylocations[0].name
        if name in in_names:
            shapes[name] = tuple(alloc.tensor_shape)

    dev = {
        "sharded": sharded, "in_names": in_names, "zero_outs": zero_outs,
        "in_shapes": shapes,
    }

    # warm-up: compile + load + one dispatch with zeros
    args = [np.zeros((N_CORES * shapes[n][0],) + tuple(shapes[n][1:]), np.float32)
            for n in in_names]
    zo = [np.zeros((N_CORES * sh[0],) + tuple(sh[1:]), dt) for sh, dt in zero_outs]
    res = sharded(*args, *zo)
    np.asarray(res[0])
    return dev


def _run_device(in_maps):
    """in_maps: list of 8 dicts name->array. Returns [8, 4, S] scores."""
    dev = _DEV
    args = [np.concatenate([in_maps[c][n] for c in range(N_CORES)], axis=0)
            for n in dev["in_names"]]
    zo = [np.zeros((N_CORES * sh[0],) + tuple(sh[1:]), dt)
          for sh, dt in dev["zero_outs"]]
    res = dev["sharded"](*args, *zo)
    out = np.asarray(res[0])                    # [8*1, 4*S]
    return out.reshape(N_CORES, 4, S)


if not os.environ.get("KERNEL_NO_DEVICE"):
    try:
        _DEV = _init_device()
    except Exception:
        _DEV = None

# ---------------------------------------------------------------------------
# host-side packing
# ---------------------------------------------------------------------------

_PERM = np.concatenate([
    np.arange(0, 256), np.arange(256, 512), np.arange(768, 1024),
    np.arange(512, 768),
])  # torch gate order [i,f,g,o] -> [i,f,o,g]


def _pack_weights(w_ih, b, w_hh):
    Wp = np.zeros((384, 1024), np.float32)
    Wp[:256] = w_ih.T[:, _PERM]
    Wp[256] = b[_PERM]
    Whp = w_hh.T[:, _PERM].astype(np.float32)
    # tiles [128,128], k-major then m: col ((d*K + k)*8 + m)*128 built per dir
    wih = Wp.reshape(3, 128, 8, 128).transpose(1, 0, 2, 3).reshape(128, 3 * 8 * 128)
    whh = Whp.reshape(2, 128, 8, 128).transpose(1, 0, 2, 3).reshape(128, 2 * 8 * 128)
    return np.ascontiguousarray(wih), np.ascontiguousarray(whh)


def _sigmoid(x):
    return 1.0 / (1.0 + np.exp(-x))


def kernel(**inputs):
    x = np.asarray(inputs["x"]).astype(np.int64)
    mask = np.asarray(inputs["mask"]).astype(bool)
    embed_table = np.asarray(inputs["embed_table"], dtype=np.float32)
    w_ih_f = np.asarray(inputs["w_ih_f"], dtype=np.float32)
    w_hh_f = np.asarray(inputs["w_hh_f"], dtype=np.float32)
    b_f = np.asarray(inputs["b_f"], dtype=np.float32)
    w_ih_b = np.asarray(inputs["w_ih_b"], dtype=np.float32)
    w_hh_b = np.asarray(inputs["w_hh_b"], dtype=np.float32)
    b_b = np.asarray(inputs["b_b"], dtype=np.float32)
    z_w = np.asarray(inputs["z_w"], dtype=np.float32)
    z_b = np.float32(np.asarray(inputs["z_b"]))

    lengths = mask.sum(1).astype(np.int64)

    probs = None
    if _DEV is not None:
        try:
            probs = _device_probs(x, mask, lengths, embed_table, w_ih_f, w_hh_f,
                                  b_f, w_ih_b, w_hh_b, b_b, z_w, z_b)
        except Exception:
            probs = None
    if probs is None:
        probs = _host_probs(x, mask, lengths, embed_table, w_ih_f, w_hh_f, b_f,
                            w_ih_b, w_hh_b, b_b, z_w, z_b)

    probs = np.where(mask, probs, 0.0).astype(np.float32)
    k = np.round(BUDGET / 100.0 * lengths.astype(np.float32)).astype(np.int64)
    ranks = np.argsort(np.argsort(-probs, axis=1, kind="stable"), axis=1, kind="stable")
    z = ((ranks < k[:, None]) & (probs > 0)).astype(np.float32)
    z = np.where(mask, z, 0.0).astype(np.float32)
    return z


def _device_probs(x, mask, lengths, embed_table, w_ih_f, w_hh_f, b_f,
                  w_ih_b, w_hh_b, b_b, z_w, z_b):
    emb = embed_table[x]                        # [B, S, E]
    emb[~mask] = 0.0
    # embT per core: [128, 2*2S]; (c, k, p, s, t)
    embT = np.ascontiguousarray(
        emb.reshape(N_CORES, 2, S, 2, 128).transpose(0, 4, 3, 1, 2)
    ).reshape(N_CORES * 128, 2 * 2 * S)

    t = np.arange(S)
    vm = (t[None, :] < lengths[:, None]).astype(np.float32)  # [B, S]
    vmask = vm.reshape(N_CORES, 1, 2 * S)

    wih_f, whh_f = _pack_weights(w_ih_f, b_f, w_hh_f)
    wih_b, whh_b = _pack_weights(w_ih_b, b_b, w_hh_b)
    wih = np.concatenate([wih_f, wih_b], axis=1)
    whh = np.concatenate([whh_f, whh_b], axis=1)
    zvec = np.ascontiguousarray(
        np.stack([z_w[0:128], z_w[128:256], z_w[256:384], z_w[384:512]], axis=1)
    ).astype(np.float32)

    in_maps = []
    for c in range(N_CORES):
        in_maps.append({
            "emb": embT[c * 128:(c + 1) * 128],
            "vmask": vmask[c],
            "wih": wih, "whh": whh, "zvec": zvec,
        })
    scores = _run_device(in_maps)               # [8, 4, S]
    sc = scores.reshape(N_CORES, 2, 2, S)       # [c, dir, s, S]
    score = sc[:, 0] + sc[:, 1]                 # [c, s, S]
    score = score.reshape(B, S) + z_b
    return _sigmoid(score.astype(np.float32))


def _host_probs(x, mask, lengths, embed_table, w_ih_f, w_hh_f, b_f,
                w_ih_b, w_hh_b, b_b, z_w, z_b):
    emb = embed_table[x]
    xp_f = emb @ w_ih_f.T + b_f
    xp_b = emb @ w_ih_b.T + b_b

    h_f = _scan(xp_f, np.ascontiguousarray(w_hh_f.T), reverse=False)

    shift = (S - lengths)
    rows = np.arange(S)[None, :]
    src = rows - shift[:, None]
    src_c = np.clip(src, 0, S - 1)
    gather_idx = src_c[:, :, None]
    xp_b_shifted = np.take_along_axis(
        xp_b, np.broadcast_to(gather_idx, xp_b.shape), axis=1)
    xp_b_shifted = np.where((src >= 0)[:, :, None], xp_b_shifted, 0.0).astype(np.float32)
    h_b_shifted = _scan(xp_b_shifted, np.ascontiguousarray(w_hh_b.T), reverse=True)
    dst = rows + shift[:, None]
    dst_c = np.clip(dst, 0, S - 1)
    h_b = np.take_along_axis(
        h_b_shifted, np.broadcast_to(dst_c[:, :, None], h_b_shifted.shape), axis=1)
    h_b = np.where((dst < S)[:, :, None], h_b, 0.0).astype(np.float32)

    scores = h_f @ z_w[:H] + h_b @ z_w[H:] + z_b
    return _sigmoid(scores.astype(np.float32))


def _scan(xp, w_hh_T, reverse):
    Bn, Sn, _ = xp.shape
    h = np.zeros((Bn, H), np.float32)
    c = np.zeros((Bn, H), np.float32)
    hs = np.empty((Bn, Sn, H), np.float32)
    order = range(Sn - 1, -1, -1) if reverse else range(Sn)
    for t in order:
        gates = xp[:, t, :] + h @ w_hh_T
        i = _sigmoid(gates[:, 0:H])
        f = _sigmoid(gates[:, H:2 * H])
        g = np.tanh(gates[:, 2 * H:3 * H])
        o = _sigmoid(gates[:, 3 * H:4 * H])
        c = f * c + i * g
        h = o * np.tanh(c)
        hs[:, t, :] = h
    return hs


# revision 6
# speedup vs baseline: 17.9983x; 3.5728x over previous
"""Trainium2 kernel for nn_BernoulliIndependentGenerator.

Full-device pipeline: per-core Bass program computes input projections,
the BiLSTM recurrence (fwd+bwd in one 1024-step hardware loop), and the
gate-score dot products for 2 samples; 8 cores cover B=16 (data
parallel over batch, per the sharding hint). Host does the embedding
gather, input packing, and the final sigmoid + per-row top-k.

Heavy one-time setup (imports, Bass trace, NEFF compile, executable
load, warm-up dispatch) happens at module import; kernel() itself only
packs inputs, runs one dispatch, and post-processes.
"""

import os
import numpy as np

B, S, E, H, V = 16, 1024, 256, 256, 50257
BUDGET = 10
N_CORES = 8
FOUR_H = 4 * H

# ---------------------------------------------------------------------------
# device setup (import time)
# ---------------------------------------------------------------------------

_DEV = None


def _init_device():
    import jax
    from jax.sharding import Mesh, PartitionSpec
    from jax.experimental.shard_map import shard_map
    import concourse.mybir as mybir
    from concourse.bass2jax import (
        install_neuronx_cc_hook, _bass_exec_p, partition_id_tensor,
    )
    from lstm_bass import build_core

    install_neuronx_cc_hook()
    nc = build_core(S)

    pname = nc.partition_id_tensor.name if nc.partition_id_tensor else None
    in_names, out_names, out_avals, zero_outs = [], [], [], []
    for alloc in nc.m.functions[0].allocations:
        if not isinstance(alloc, mybir.MemoryLocationSet):
            continue
        name = alloc.memorylocations[0].name
        if alloc.kind == "ExternalInput":
            if name != pname:
                in_names.append(name)
        elif alloc.kind == "ExternalOutput":
            out_names.append(name)
            shape = tuple(alloc.tensor_shape)
            dtype = mybir.dt.np(alloc.dtype)
            out_avals.append(jax.core.ShapedArray(shape, dtype))
            zero_outs.append((shape, dtype))
    n_params = len(in_names)
    n_outs = len(out_avals)
    in_names_all = list(in_names) + out_names + ([pname] if pname else [])
    donate = tuple(range(n_params, n_params + n_outs))

    def _body(*args):
        operands = list(args)
        if pname is not None:
            operands.append(partition_id_tensor())
        outs = _bass_exec_p.bind(
            *operands, out_avals=tuple(out_avals), in_names=tuple(in_names_all),
            out_names=tuple(out_names), lowering_input_output_aliases=(),
            sim_require_finite=False, sim_require_nnan=False, nc=nc)
        return tuple(outs)

    devices = jax.devices()[:N_CORES]
    mesh = Mesh(np.asarray(devices), ("core",))
    sharded = jax.jit(
        shard_map(_body, mesh=mesh,
                  in_specs=(PartitionSpec("core"),) * (n_params + n_outs),
                  out_specs=(PartitionSpec("core"),) * n_outs, check_rep=False),
        donate_argnums=donate, keep_unused=True)

    shapes = {}
    for alloc in nc.m.functions[0].allocations:
        if not isinstance(alloc, mybir.MemoryLocationSet):
            continue
        name = alloc.memorylocations[0].name
        if name in in_names:
            shapes[name] = tuple(alloc.tensor_shape)

    dev = {
        "sharded": sharded, "in_names": in_names, "zero_outs": zero_outs,
        "in_shapes": shapes,
    }

    # warm-up: compile + load + one dispatch with zeros
    args = [np.zeros((N_CORES * shapes[n][0],) + tuple(shapes[n][1:]), np.float32)
            for n in in_names]
    zo = [np.zeros((N_CORES * sh[0],) + tuple(sh[1:]), dt) for sh, dt in zero_outs]
    res = sharded(*args, *zo)
    np.asarray(res[0])
    return dev


def _run_device(globals_by_name):
    """globals_by_name: name -> global [8*rows, cols] array. Returns [8,4,S]."""
    dev = _DEV
    args = [globals_by_name[n] for n in dev["in_names"]]
    zo = [np.zeros((N_CORES * sh[0],) + tuple(sh[1:]), dt)
          for sh, dt in dev["zero_outs"]]
    res = dev["sharded"](*args, *zo)
    out = np.asarray(res[0])                    # [8*1, 4*S]
    return out.reshape(N_CORES, 4, S)


if not os.environ.get("KERNEL_NO_DEVICE"):
    try:
        _DEV = _init_device()
    except Exception:
        _DEV = None

# ---------------------------------------------------------------------------
# host-side packing
# ---------------------------------------------------------------------------

_PERM = np.concatenate([
    np.arange(0, 256), np.arange(256, 512), np.arange(768, 1024),
    np.arange(512, 768),
])  # torch gate order [i,f,g,o] -> [i,f,o,g]


def _pack_weights(w_ih, b, w_hh):
    Wp = np.zeros((384, 1024), np.float32)
    Wp[:256] = w_ih.T[:, _PERM]
    Wp[256] = b[_PERM]
    Whp = w_hh.T[:, _PERM].astype(np.float32)
    # tiles [128,128], k-major then m: col ((d*K + k)*8 + m)*128 built per dir
    wih = Wp.reshape(3, 128, 8, 128).transpose(1, 0, 2, 3).reshape(128, 3 * 8 * 128)
    whh = Whp.reshape(2, 128, 8, 128).transpose(1, 0, 2, 3).reshape(128, 2 * 8 * 128)
    return np.ascontiguousarray(wih), np.ascontiguousarray(whh)


def _sigmoid(x):
    return 1.0 / (1.0 + np.exp(-x))


def kernel(**inputs):
    x = np.asarray(inputs["x"]).astype(np.int64)
    mask = np.asarray(inputs["mask"]).astype(bool)
    embed_table = np.asarray(inputs["embed_table"], dtype=np.float32)
    w_ih_f = np.asarray(inputs["w_ih_f"], dtype=np.float32)
    w_hh_f = np.asarray(inputs["w_hh_f"], dtype=np.float32)
    b_f = np.asarray(inputs["b_f"], dtype=np.float32)
    w_ih_b = np.asarray(inputs["w_ih_b"], dtype=np.float32)
    w_hh_b = np.asarray(inputs["w_hh_b"], dtype=np.float32)
    b_b = np.asarray(inputs["b_b"], dtype=np.float32)
    z_w = np.asarray(inputs["z_w"], dtype=np.float32)
    z_b = np.float32(np.asarray(inputs["z_b"]))

    lengths = mask.sum(1).astype(np.int64)

    probs = None
    if _DEV is not None:
        try:
            probs = _device_probs(x, mask, lengths, embed_table, w_ih_f, w_hh_f,
                                  b_f, w_ih_b, w_hh_b, b_b, z_w, z_b)
        except Exception:
            probs = None
    if probs is None:
        probs = _host_probs(x, mask, lengths, embed_table, w_ih_f, w_hh_f, b_f,
                            w_ih_b, w_hh_b, b_b, z_w, z_b)

    probs = np.where(mask, probs, 0.0).astype(np.float32)
    k = np.round(BUDGET / 100.0 * lengths.astype(np.float32)).astype(np.int64)
    ranks = np.argsort(np.argsort(-probs, axis=1, kind="stable"), axis=1, kind="stable")
    z = ((ranks < k[:, None]) & (probs > 0)).astype(np.float32)
    z = np.where(mask, z, 0.0).astype(np.float32)
    return z


def _device_probs(x, mask, lengths, embed_table, w_ih_f, w_hh_f, b_f,
                  w_ih_b, w_hh_b, b_b, z_w, z_b):
    emb = embed_table[x]                        # [B, S, E]
    emb[~mask] = 0.0
    # embT per core: [128, 2*2S]; (c, k, p, s, t)
    embT = np.ascontiguousarray(
        emb.reshape(N_CORES, 2, S, 2, 128).transpose(0, 4, 3, 1, 2)
    ).reshape(N_CORES * 128, 2 * 2 * S)

    t = np.arange(S)
    vm = (t[None, :] < lengths[:, None]).astype(np.float32)  # [B, S]
    vmask = vm.reshape(N_CORES, 1, 2 * S)

    wih_f, whh_f = _pack_weights(w_ih_f, b_f, w_hh_f)
    wih_b, whh_b = _pack_weights(w_ih_b, b_b, w_hh_b)
    wih = np.concatenate([wih_f, wih_b], axis=1)
    whh = np.concatenate([whh_f, whh_b], axis=1)
    zvec = np.ascontiguousarray(
        np.stack([z_w[0:128], z_w[128:256], z_w[256:384], z_w[384:512]], axis=1)
    ).astype(np.float32)

    zvec_g = np.empty((N_CORES * 128, 4), np.float32)
    zvec_g.reshape(N_CORES, 128, 4)[:] = zvec

    # wih/whh are 1/8-partition-sharded inputs; the global array is just the
    # packed [128, cols] matrix itself (core c takes rows 16c:16c+16) and the
    # kernel AllGathers on device.
    scores = _run_device({
        "emb": embT, "vmask": vmask.reshape(N_CORES, 2 * S),
        "wih": wih, "whh": whh, "zvec": zvec_g,
    })                                          # [8, 4, S]
    sc = scores.reshape(N_CORES, 2, 2, S)       # [c, dir, s, S]
    score = sc[:, 0] + sc[:, 1]                 # [c, s, S]
    score = score.reshape(B, S) + z_b
    return _sigmoid(score.astype(np.float32))


def _host_probs(x, mask, lengths, embed_table, w_ih_f, w_hh_f, b_f,
                w_ih_b, w_hh_b, b_b, z_w, z_b):
    emb = embed_table[x]
    xp_f = emb @ w_ih_f.T + b_f
    xp_b = emb @ w_ih_b.T + b_b

    h_f = _scan(xp_f, np.ascontiguousarray(w_hh_f.T), reverse=False)

    shift = (S - lengths)
    rows = np.arange(S)[None, :]
    src = rows - shift[:, None]
    src_c = np.clip(src, 0, S - 1)
    gather_idx = src_c[:, :, None]
    xp_b_shifted = np.take_along_axis(
        xp_b, np.broadcast_to(gather_idx, xp_b.shape), axis=1)
    xp_b_shifted = np.where((src >= 0)[:, :, None], xp_b_shifted, 0.0).astype(np.float32)
    h_b_shifted = _scan(xp_b_shifted, np.ascontiguousarray(w_hh_b.T), reverse=True)
    dst = rows + shift[:, None]
    dst_c = np.clip(dst, 0, S - 1)
    h_b = np.take_along_axis(
        h_b_shifted, np.broadcast_to(dst_c[:, :, None], h_b_shifted.shape), axis=1)
    h_b = np.where((dst < S)[:, :, None], h_b, 0.0).astype(np.float32)

    scores = h_f @ z_w[:H] + h_b @ z_w[H:] + z_b
    return _sigmoid(scores.astype(np.float32))


def _scan(xp, w_hh_T, reverse):
    Bn, Sn, _ = xp.shape
    h = np.zeros((Bn, H), np.float32)
    c = np.zeros((Bn, H), np.float32)
    hs = np.empty((Bn, Sn, H), np.float32)
    order = range(Sn - 1, -1, -1) if reverse else range(Sn)
    for t in order:
        gates = xp[:, t, :] + h @ w_hh_T
        i = _sigmoid(gates[:, 0:H])
        f = _sigmoid(gates[:, H:2 * H])
        g = np.tanh(gates[:, 2 * H:3 * H])
        o = _sigmoid(gates[:, 3 * H:4 * H])
        c = f * c + i * g
        h = o * np.tanh(c)
        hs[:, t, :] = h
    return hs
